# revision 8
# baseline (speedup 1.0000x reference)
"""CLALoss Trainium2 kernel.

Strategy (data-parallel over the batch, per the sharding hint):
  - Host shards the B=16384 (user, item) batch across 8 NeuronCores
    (2048 samples each) and extracts each shard's embedding rows
    (transposed to [D, B_loc] so the device matmuls need no transposes).
  - Each core (identical SPMD program):
      * l2-normalizes the two K x D prototype heads (replicated),
      * computes the 5 code matrices [B_loc, K] via TensorE,
        folding the per-row 1/|x| l2-norm scaling into the PSUM eviction,
      * runs 3 Sinkhorn-Knopp iterations in diagonal-scaling form
        q = Q0 * (alpha x beta):  alpha = 1/colsum(Q0 * beta) needs a
        global (all-B) sum -> one 5x256 AllReduce per iteration (3 total),
        beta = 1/rowsum(Q0 * alpha) is shard-local,
      * computes log_softmax(code/g) in place and the 8 cross-entropy
        partial dot products sum(q * lsm) for its shard.
  - Host sums the 8x[8] partials and applies the 1/2, 1/6, -1/B weights.

Sinkhorn scale-invariance: the global sum(Q) normalization and the K/B
factors cancel in q, so only the 3 column-sum AllReduces are needed.
"""

import sys

if "/opt/trn_rl_repo" not in sys.path:
    sys.path.insert(0, "/opt/trn_rl_repo")

import numpy as np

N_CORES = 8
B = 16384
B_LOC = B // N_CORES          # 2048
T = B_LOC // 128              # 16 partition-tiles per core
D = 64
K = 256
INV_EPS = 20.0                # 1/0.05 sinkhorn temperature

# CE pairs (q_code_idx, lsm_code_idx); codes: 0=user 1=id 2=id_ii 3=im_ii 4=tx_ii
PAIRS = [(0, 1), (1, 0), (2, 3), (2, 4), (3, 2), (3, 4), (4, 2), (4, 3)]

_CACHE = {}


def _build_nc():
    import concourse.bass as bass
    import concourse.bacc as bacc
    import concourse.tile as tile
    from concourse import mybir

    f32 = mybir.dt.float32
    bf16 = mybir.dt.bfloat16
    AFT = mybir.ActivationFunctionType
    ALU = mybir.AluOpType

    nc = bacc.Bacc("TRN2", target_bir_lowering=False, debug=False,
                   num_devices=N_CORES)

    xts_d = [
        nc.dram_tensor(name, [D, B_LOC], f32, kind="ExternalInput")
        for name in ("ut", "idt", "imt", "txt")
    ]
    wft = nc.dram_tensor("wft", [D, K], f32, kind="ExternalInput")
    wit = nc.dram_tensor("wit", [D, K], f32, kind="ExternalInput")
    gin = nc.dram_tensor("gin", [128, 1], f32, kind="ExternalInput")
    partials = nc.dram_tensor("partials", [8, 1], f32, kind="ExternalOutput")

    with tile.TileContext(nc) as tc:
        with (
            tc.tile_pool(name="sb_main", bufs=1) as sbm,
            tc.tile_pool(name="sb_small", bufs=1) as sbs,
            tc.tile_pool(name="dram", bufs=2, space="DRAM") as dram,
        ):
            # ---- constants ----
            ones_f32 = sbs.tile([128, 1], f32)
            nc.vector.memset(ones_f32[:], 1.0)
            ones_bf = sbs.tile([128, 1], bf16)
            nc.vector.memset(ones_bf[:], 1.0)
            ones_row = sbs.tile([1, 128], f32)
            nc.vector.memset(ones_row[:], 1.0)

            gsb = sbs.tile([128, 1], f32)
            nc.sync.dma_start(gsb[:], gin[:, :])
            gclip = sbs.tile([128, 1], f32)
            nc.vector.tensor_scalar(gclip[:], gsb[:], 0.01, 0.99,
                                    op0=ALU.max, op1=ALU.min)
            invg = sbs.tile([128, 1], f32)
            nc.vector.reciprocal(invg[:], gclip[:])
            invg_neg = sbs.tile([128, 1], f32)
            nc.vector.tensor_scalar_mul(invg_neg[:], invg[:], -1.0)

            # ---- big SBUF tensors ----
            codes = [sbm.tile([128, T * K], f32, name=f"code{c}")
                     for c in range(5)]
            q0s = [sbm.tile([128, T * K], bf16, name=f"q0_{c}")
                   for c in range(5)]

            with (
                tc.tile_pool(name="sb_in", bufs=1) as sbi,
                tc.tile_pool(name="ppc", bufs=4, space="PSUM") as ppc,
                tc.tile_pool(name="ppn", bufs=1, space="PSUM") as ppn,
            ):
                # ---- phase A: normalize prototype heads ----
                wsb = sbi.tile([D, 2 * K], f32)
                nc.sync.dma_start(wsb[:, 0:K], wft[:, :])
                nc.sync.dma_start(wsb[:, K:2 * K], wit[:, :])
                sqw = sbi.tile([D, 2 * K], f32)
                nc.scalar.square(sqw[:], wsb[:])
                pw = ppn.tile([1, 2 * K], f32)
                nc.tensor.matmul(pw[:], ones_f32[0:D, :], sqw[:],
                                 start=True, stop=True)
                wrec = sbi.tile([1, 2 * K], f32)
                nc.vector.reciprocal(wrec[:], pw[:])
                winv = sbi.tile([1, 2 * K], f32)
                nc.scalar.sqrt(winv[:], wrec[:])
                pwb = ppn.tile([D, 2 * K], f32)
                nc.tensor.matmul(pwb[:], ones_row[:, 0:D], winv[:],
                                 start=True, stop=True)
                wn = sbi.tile([D, 2 * K], f32)
                nc.vector.tensor_mul(wn[:], wsb[:], pwb[:])

                # ---- phase B: embedding row norms ----
                xts = []
                for m, xd in enumerate(xts_d):
                    xt = sbi.tile([D, B_LOC], f32, name=f"xt{m}")
                    nc.sync.dma_start(xt[:], xd[:, :])
                    xts.append(xt)
                pn = ppn.tile([128, 4 * T], f32)
                for m in range(4):
                    sq = sbi.tile([D, B_LOC], f32, tag="sq")
                    nc.scalar.square(sq[:], xts[m][:])
                    for t in range(T):
                        nc.tensor.matmul(
                            pn[:, m * T + t:m * T + t + 1],
                            sq[:, t * 128:(t + 1) * 128],
                            ones_f32[0:D, :],
                            start=True, stop=True)
                nrec = sbi.tile([128, 4 * T], f32)
                nc.vector.reciprocal(nrec[:], pn[:])
                invu = sbs.tile([128, 4 * T], f32)
                nc.scalar.sqrt(invu[:], nrec[:])

                # ---- phase C: code matmuls, evict with 1/|x| row scaling ----
                # (modality, wn column slice, [(code idx, psum col offset)...])
                plan = [
                    (0, (0, K), [(0, 0)]),
                    (1, (0, 2 * K), [(1, 0), (2, K)]),
                    (2, (K, 2 * K), [(3, 0)]),
                    (3, (K, 2 * K), [(4, 0)]),
                ]
                for t in range(T):
                    for m, (w0, w1), dests in plan:
                        wcols = w1 - w0
                        pc = ppc.tile([128, 512], f32, tag="pc")
                        nc.tensor.matmul(
                            pc[:, 0:wcols],
                            xts[m][:, t * 128:(t + 1) * 128],
                            wn[:, w0:w1],
                            start=True, stop=True)
                        for cidx, off in dests:
                            nc.vector.tensor_scalar_mul(
                                codes[cidx][:, t * K:(t + 1) * K],
                                pc[:, off:off + K],
                                invu[:, m * T + t:m * T + t + 1])

            # ---- phase D: Q0 = exp(code * 20) (bf16) ----
            for c in range(5):
                nc.scalar.activation(q0s[c][:], codes[c][:], AFT.Exp,
                                     scale=INV_EPS)

            with (
                tc.tile_pool(name="sb_it", bufs=1) as sbt,
                tc.tile_pool(name="ppr", bufs=2, space="PSUM") as ppr,
                tc.tile_pool(name="ppab", bufs=2, space="PSUM") as ppab,
                tc.tile_pool(name="ppexp", bufs=2, space="PSUM") as ppexp,
            ):
                prod = None  # lazily allocated DVE product scratch
                # ---- phase F (emitted early; runs on ACT/DVE while the
                #      sinkhorn AllReduces are in flight):
                #      lsm = code*invg - (m*invg + ln s), in place ----
                for c in range(5):
                    m_sb = sbt.tile([128, T], f32, name=f"m{c}")
                    nc.vector.tensor_reduce(
                        m_sb[:],
                        codes[c].rearrange("p (t k) -> p t k", k=K),
                        axis=mybir.AxisListType.X, op=ALU.max)
                    bias_sb = sbt.tile([128, T], f32, name=f"bias{c}")
                    nc.vector.tensor_scalar(bias_sb[:], m_sb[:],
                                            invg_neg[:], None, op0=ALU.mult)
                    s_sb = sbt.tile([128, T], f32, name=f"se{c}")
                    for t in range(T):
                        pe = ppexp.tile([128, K], f32, tag="pe")
                        nc.scalar.activation(
                            pe[:], codes[c][:, t * K:(t + 1) * K], AFT.Exp,
                            bias=bias_sb[:, t:t + 1], scale=invg[:],
                            accum_out=s_sb[:, t:t + 1])
                    lns = sbt.tile([128, T], f32, name=f"lns{c}")
                    nc.scalar.activation(lns[:], s_sb[:], AFT.Ln)
                    mm = sbt.tile([128, T], f32, name=f"mm{c}")
                    nc.vector.tensor_sub(mm[:], lns[:], bias_sb[:])
                    for t in range(T):
                        nc.vector.tensor_scalar(
                            codes[c][:, t * K:(t + 1) * K],
                            codes[c][:, t * K:(t + 1) * K],
                            invg[:], mm[:, t:t + 1],
                            op0=ALU.mult, op1=ALU.subtract)

                # ---- phase E: 3 sinkhorn iterations ----
                betas = [None] * 5
                for it in range(3):
                    ar_in = dram.tile([5, K], f32, tag="ar_in")
                    ar_out = dram.tile([5, K], f32, tag="ar_out")
                    for c in range(5):
                        pr = ppr.tile([1, K], f32, tag="pr")
                        for t in range(T):
                            lhs = (ones_bf[:, :] if it == 0
                                   else betas[c][:, t:t + 1])
                            nc.tensor.matmul(
                                pr[:], lhs,
                                q0s[c][:, t * K:(t + 1) * K],
                                start=(t == 0), stop=(t == T - 1))
                        r_sb = sbt.tile([1, K], f32, tag="r_sb", bufs=2)
                        nc.scalar.copy(r_sb[:], pr[:])
                        nc.sync.dma_start(ar_in[c:c + 1, :], r_sb[:])
                    nc.gpsimd.collective_compute(
                        "AllReduce", ALU.add,
                        replica_groups=[list(range(N_CORES))],
                        ins=[ar_in.opt()], outs=[ar_out.opt()])
                    for c in range(5):
                        rrow = sbt.tile([1, K], f32, tag="rrow", bufs=2)
                        nc.sync.dma_start(rrow[:], ar_out[c:c + 1, :])
                        rrec = sbt.tile([1, K], f32, tag="rrec", bufs=2)
                        nc.vector.reciprocal(rrec[:], rrow[:])
                        pab = ppab.tile([128, K], f32, tag="pab")
                        nc.tensor.matmul(pab[:], ones_row[:, :],
                                         rrec[:, :],
                                         start=True, stop=True)
                        ab = sbt.tile([128, K], f32, tag="ab", bufs=2)
                        nc.scalar.copy(ab[:], pab[:])
                        s_it = sbt.tile([128, T], f32, tag=f"s{c}", bufs=2)
                        if it < 2:
                            if prod is None:
                                prod = sbt.tile([128, T * K], f32,
                                                name="prod")
                            for t in range(T):
                                nc.vector.tensor_mul(
                                    prod[:, t * K:(t + 1) * K],
                                    q0s[c][:, t * K:(t + 1) * K],
                                    ab[:])
                            nc.vector.tensor_reduce(
                                s_it[:],
                                prod.rearrange("p (t k) -> p t k", k=K),
                                axis=mybir.AxisListType.X, op=ALU.add)
                            bf = sbt.tile([128, T], f32, tag=f"bf{c}", bufs=2)
                            nc.vector.reciprocal(bf[:], s_it[:])
                            bb = sbt.tile([128, T], bf16, tag=f"bb{c}", bufs=2)
                            nc.vector.tensor_copy(bb[:], bf[:])
                            betas[c] = bb
                        else:
                            for t in range(T):
                                nc.vector.tensor_mul(
                                    q0s[c][:, t * K:(t + 1) * K],
                                    q0s[c][:, t * K:(t + 1) * K],
                                    ab[:])
                            nc.vector.tensor_reduce(
                                s_it[:],
                                q0s[c].rearrange("p (t k) -> p t k", k=K),
                                axis=mybir.AxisListType.X, op=ALU.add)
                            bf = sbt.tile([128, T], f32, tag=f"bf{c}", bufs=2)
                            nc.vector.reciprocal(bf[:], s_it[:])
                            for t in range(T):
                                nc.vector.tensor_scalar_mul(
                                    q0s[c][:, t * K:(t + 1) * K],
                                    q0s[c][:, t * K:(t + 1) * K],
                                    bf[:, t:t + 1])

                # ---- phase G: 8 CE partial dots sum(q * lsm) ----
                dcol = sbt.tile([128, 8], f32)
                for j, (a, b) in enumerate(PAIRS):
                    nc.vector.tensor_mul(prod[:], q0s[a][:], codes[b][:])
                    nc.vector.tensor_reduce(dcol[:, j:j + 1], prod[:],
                                            axis=mybir.AxisListType.X,
                                            op=ALU.add)
                pdot = ppr.tile([8, 1], f32, tag="pr")
                nc.tensor.matmul(pdot[:], dcol[:], ones_f32[:],
                                 start=True, stop=True)
                out_sb = sbt.tile([8, 1], f32)
                nc.scalar.copy(out_sb[:], pdot[:])
                nc.sync.dma_start(partials[:, :], out_sb[:])

    nc.compile()
    return nc


def _get_nc():
    if "nc" not in _CACHE:
        _CACHE["nc"] = _build_nc()
    return _CACHE["nc"]


def _prepare_in_maps(user_embeddings, id_embeddings, image_embeddings,
                     text_embeddings, users, items, feat2code_w, ii2code_w,
                     gamma):
    users = np.asarray(users).astype(np.int64)
    items = np.asarray(items).astype(np.int64)
    gathered = [
        np.asarray(user_embeddings, np.float32)[users],
        np.asarray(id_embeddings, np.float32)[items],
        np.asarray(image_embeddings, np.float32)[items],
        np.asarray(text_embeddings, np.float32)[items],
    ]
    wftT = np.ascontiguousarray(np.asarray(feat2code_w, np.float32).T)
    witT = np.ascontiguousarray(np.asarray(ii2code_w, np.float32).T)
    grep = np.full((128, 1), np.float32(gamma), np.float32)
    names = ("ut", "idt", "imt", "txt")
    in_maps = []
    for c in range(N_CORES):
        sl = slice(c * B_LOC, (c + 1) * B_LOC)
        m = {name: np.ascontiguousarray(g[sl].T)
             for name, g in zip(names, gathered)}
        m["wft"] = wftT
        m["wit"] = witT
        m["gin"] = grep
        in_maps.append(m)
    return in_maps


def _combine(results):
    P = np.zeros(8, np.float64)
    for c in range(N_CORES):
        P += results[c]["partials"][:, 0].astype(np.float64)
    loss = -((P[0] + P[1]) / 2.0 + (P[2] + P[3] + P[4] + P[5] + P[6] + P[7])
             / 6.0) / B
    return np.asarray(loss, dtype=np.float32)


def run(trace=False, **inputs):
    from concourse.bass_utils import run_bass_kernel_spmd

    nc = _get_nc()
    in_maps = _prepare_in_maps(**inputs)
    res = run_bass_kernel_spmd(nc, in_maps, list(range(N_CORES)),
                               trace=trace)
    return _combine(res.results), res


def kernel(**inputs):
    out, _ = run(trace=False, **inputs)
    return out
